# revision 1
# baseline (speedup 1.0000x reference)
"""HazardRNN Trainium2 kernel.

Math (per batch lane n, hidden unit j):
    h_{t}[j,n] = tanh(W_in[j] * x[n,t] + b_in[j] + h_{t-1}[j,n]),  t = 0..S-1
    out[n]     = softmax(h_{S-1} @ W_out + b_out)

Sharding: hidden dim (800) split over 8 cores (100 each). Every core sees the
full batch (256 lanes, processed as 2 independent halves of 128).

Per-core layout ("E-layout"): partitions = hidden row (1..100), free = batch.
Each step of the recurrence is ONE fp32 matmul + ONE scalar-engine tanh:

  stationary M [128,101]:  M[0, 1+q] = W_in[j0+q]   (x broadcast row, w-scaled)
                           M[1+k, 1+q] = (k==q)     (identity pass-through of h)
  moving rhs  [128, 128]:  row 0 = x_t for 128 lanes, rows 1..100 = h_{t-1}
  psum[1+q, n] = W_in[q]*x_t[n] + h_{t-1}[q, n]
  ACT: h_t = tanh(psum * 1 + b_col)   (per-partition bias adds b_in)

h_t is written by the activation directly into the *next* ring position, so the
moving operand of the next matmul is ready without any copies. Ring row 0 holds
x (pre-staged t-major by background DMA, CHUNK steps per refill).

The final projection is one matmul with lhsT = W_out slice [128, 2] over the
final h; each core DMAs out its partial logits [2, 256]. The host sums the 8
partials, adds b_out and applies a (tiny, 256x2) softmax.
"""

import numpy as np

S = 1024
NB = 256  # B*E batch lanes
HIDDEN = 800
NCORES = 8
HPC = HIDDEN // NCORES  # hidden rows per core = 100
BH = 128  # batch half
NH = NB // BH  # 2 halves
CHUNK = 64  # ring positions per x-refill DMA

_CACHE: dict = {}


def _build_nc(s_steps: int):
    import concourse.bass as bass
    import concourse.mybir as mybir
    from concourse.tile import TileContext

    f32 = mybir.dt.float32
    AF = mybir.ActivationFunctionType
    nchunks = s_steps // CHUNK
    assert s_steps % CHUNK == 0

    nc = bass.Bass()
    xT = nc.declare_dram_parameter("xT", [s_steps, NB], f32, isOutput=False)
    Md = nc.declare_dram_parameter("M", [128, HPC], f32, isOutput=False)
    bd = nc.declare_dram_parameter("bcol", [128, 1], f32, isOutput=False)
    wd = nc.declare_dram_parameter("woE", [128, 2], f32, isOutput=False)
    zd = nc.declare_dram_parameter("zinit", [HPC, BH], f32, isOutput=False)
    outd = nc.declare_dram_parameter("partial", [2, NB], f32, isOutput=True)

    with TileContext(nc) as tc:
        with (
            tc.tile_pool(name="const", bufs=1) as cp,
            tc.tile_pool(name="ring", bufs=1) as rp,
            tc.tile_pool(name="ps", bufs=5, space="PSUM") as pp,
            tc.tile_pool(name="ps_fin", bufs=2, space="PSUM") as pp2,
            tc.tile_pool(name="fin", bufs=1) as fp,
        ):
            Mt = cp.tile([128, HPC], f32, tag="Mt")
            nc.sync.dma_start(out=Mt[:], in_=Md[:])
            bt = cp.tile([128, 1], f32, tag="bt")
            nc.sync.dma_start(out=bt[:], in_=bd[:])
            wt = cp.tile([128, 2], f32, tag="wt")
            nc.sync.dma_start(out=wt[:], in_=wd[:])

            # Ring buffers: [128, CHUNK*BH] per (half, parity).
            rings = [
                [
                    rp.tile(
                        [HPC + 1, CHUNK * BH], f32,
                        name=f"ring{h}_{i}", tag=f"ring{h}_{i}",
                    )
                    for i in range(2)
                ]
                for h in range(NH)
            ]
            def dma_x(h, c):
                # load x rows for chunk c into ring[h][(c%2)] row HPC (x row)
                buf = rings[h][c % 2]
                nc.sync.dma_start(
                    out=buf[HPC : HPC + 1, :].rearrange("p (t n) -> p t n", t=CHUNK),
                    in_=xT[c * CHUNK : (c + 1) * CHUNK, h * BH : (h + 1) * BH],
                )

            for h in range(NH):
                for c in range(min(2, nchunks)):
                    dma_x(h, c)
            # zero-init the position-0 h block of the first buffer (h_0 = 0);
            # emitted after the x loads so observer ordering below works.
            for h in range(NH):
                nc.sync.dma_start(out=rings[h][0][0:HPC, 0:BH], in_=zd[:])

            # Observer matmuls: the ISA LDWEIGHTS slot carries at most ONE
            # sync wait, so every DMA-queue semaphore a real matmul would
            # need must be pre-observed by PE. Each observer is a [K,1]x[K,1]
            # matmul into its own column of a scratch PSUM tile (disjoint
            # bytes -> no WAW chain between observers).
            obs_ps = pp2.tile([1, 64], f32, name="obs_ps", tag="obs", bufs=1)
            obs_k = [0]

            def pe_observe(ap):
                base = ap.base_partition()
                nc.tensor.matmul(
                    out=obs_ps[0:1, obs_k[0] : obs_k[0] + 1],
                    lhsT=ap, rhs=ap, start=True, stop=True,
                    tile_position=(base, 0),
                )
                obs_k[0] += 1

            for h in range(NH):
                for i in range(min(2, nchunks)):
                    # absorb the x-prefill queue (x-exclusive bytes), then the
                    # zinit queue for buffer 0
                    pe_observe(rings[h][i][64 : HPC + 1, 2 * BH : 2 * BH + 1])
                pe_observe(rings[h][0][0:101, 0:1])
            pe_observe(wt[:, 0:1])
            pe_observe(Mt[:, 0:1])
            # ACT warm-up: pre-observe the bias DMA queue on the scalar engine
            scratch = cp.tile([128, 64], f32, name="scratch", tag="scratch")
            nc.scalar.activation(
                out=scratch[:, 0:1], in_=bt[:], func=AF.Tanh, bias=bt[:],
            )

            for t in range(s_steps):
                c, pos = divmod(t, CHUNK)
                nch, npos = divmod(t + 1, CHUNK)
                for h in range(NH):
                    buf = rings[h][c % 2]
                    nbuf = rings[h][nch % 2]
                    ps = pp.tile([128, BH], f32, name="ps", tag="ps")
                    nc.tensor.matmul(
                        out=ps[0:HPC, :],
                        lhsT=Mt[0 : HPC + 1, :],
                        rhs=buf[:, pos * BH : (pos + 1) * BH],
                        start=True,
                        stop=True,
                    )
                    nc.scalar.activation(
                        out=nbuf[0:HPC, npos * BH : (npos + 1) * BH],
                        in_=ps[0:HPC, :],
                        func=AF.Tanh,
                        bias=bt[0:HPC, :],
                    )
                if pos == CHUNK - 1 and c + 2 < nchunks:
                    for h in range(NH):
                        dma_x(h, c + 2)
                        pe_observe(
                            rings[h][c % 2][64 : HPC + 1, 2 * BH : 2 * BH + 1]
                        )

            # Final projection: partial logits [2, NB]. Final h sits at ring
            # position (s_steps % (2*CHUNK)) mapped to buffer/position below.
            fc, fpos = divmod(s_steps, CHUNK)
            partial = fp.tile([2, NB], f32, tag="partial")
            for h in range(NH):
                fbuf = rings[h][fc % 2]
                ps2 = pp2.tile([2, NB], f32, name="ps2", tag="ps2")
                nc.tensor.matmul(
                    out=ps2[:, 0:BH],
                    lhsT=wt[0 : HPC + 1, :],
                    rhs=fbuf[:, fpos * BH : (fpos + 1) * BH],
                    start=True,
                    stop=True,
                )
                nc.vector.tensor_copy(partial[:, h * BH : (h + 1) * BH], ps2[:, 0:BH])
            nc.sync.dma_start(out=outd[0:1, :], in_=partial[0:1, :])
            nc.sync.dma_start(out=outd[1:2, :], in_=partial[1:2, :])


    # The kernel-tail drain exceeds the ISA wait-slot limit (it waits every
    # DMA queue ever used). All in-kernel DMAs are consumed by compute that
    # the engine waits already cover; only the OUTPUT DMAs' queues must be
    # awaited for the result to land before the NEFF retires. Keep exactly
    # those queue waits plus the engine waits.
    # Refill DMAs carry {PE(WAR), ACT(WAW)} waits; a DMA has ONE ISA wait
    # slot. Every ACT(t) in this kernel waits its same-step matmul, so the
    # ACT tick transitively dominates the (strictly older) PE tick: drop PE.
    for bb in nc.m.functions[0].blocks:
        for i in bb.instructions:
            if type(i).__name__ not in ("InstDMACopy", "InstActivation"):
                continue
            si = i.sync_info
            try:
                ws = list(si.on_wait)
            except Exception:
                continue
            names = [w.ant_name for w in ws]
            pe = [w for w in ws if "PE" in w.ant_name]
            rest = [w for w in ws if "PE" not in w.ant_name]
            if len(ws) > 1 and len(pe) == 1 and all(
                "DMAHW" in n or "Activation" in n
                for n in (w.ant_name for w in rest)
            ):
                # The PE(WAR) tick covers the matmuls that consumed the
                # prior refill on this buffer, so the WAW queue waits are
                # transitively satisfied.
                si.on_wait = pe

    out_q = set()
    for bb in nc.m.functions[0].blocks:
        for i in bb.instructions:
            if type(i).__name__ == "InstDMACopy":
                try:
                    dst = i.outs[0].tensor_name
                except Exception:
                    dst = getattr(getattr(i.outs[0], "tensor", None), "name", "")
                if "partial" in str(dst) or "partial" in str(i.outs[0]):
                    si = i.sync_info
                    try:
                        for u in si.on_update:
                            out_q.add(u.ant_name)
                    except Exception:
                        pass
    for bb in nc.m.functions[0].blocks:
        insts = list(bb.instructions)
        tail_idx = None
        for idx, i in enumerate(insts):
            si = i.sync_info
            try:
                nw = len(si.on_wait)
            except Exception:
                continue
            if type(i).__name__ == "InstDrain" and nw > 3:
                tail_idx = idx
                break
        if tail_idx is None:
            continue
        drain = insts[tail_idx]
        si = drain.sync_info
        keepable = [
            w for w in si.on_wait
            if "DMAHW" not in w.ant_name or w.ant_name in out_q
        ]
        moved = keepable[1:]
        si.on_wait = keepable[:1]
        # Each drain carries at most ONE ISA wait slot: emit one extra
        # SP drain per remaining wait at the end of the main body block
        # (before the tail block's semaphore clear).
        import concourse.mybir as mybir
        blocks = list(nc.m.functions[0].blocks)
        body_bb = blocks[blocks.index(bb) - 1]
        for k, w in enumerate(moved):
            d = mybir.InstDrain(name=f"xtra_tail_drain_{k}", ins=[], outs=[])
            d.engine = mybir.EngineType.SP
            d.sync_info = type(si)(on_wait=[w], on_update=[])
            body_bb.add_instruction(d)
    return nc


def _prep_inputs(x, W_in, b_in, W_out, s_steps):
    """Host-side shard prep. Returns in_maps for run_bass_kernel_spmd."""
    x2 = np.ascontiguousarray(
        x.reshape(NB, s_steps).astype(np.float32)
    )  # [n, t] after squeeze
    xT = np.ascontiguousarray(x2.T)  # [t, n]
    w = W_in.reshape(HIDDEN).astype(np.float32)
    b = b_in.reshape(HIDDEN).astype(np.float32)
    wo = W_out.astype(np.float32)
    in_maps = []
    for core in range(NCORES):
        j0 = core * HPC
        M = np.zeros((128, HPC), np.float32)
        M[0:HPC, 0:HPC] = np.eye(HPC, dtype=np.float32)
        M[HPC, 0:HPC] = w[j0 : j0 + HPC]
        bcol = np.zeros((128, 1), np.float32)
        bcol[0:HPC, 0] = b[j0 : j0 + HPC]
        woE = np.zeros((128, 2), np.float32)
        woE[0:HPC, :] = wo[j0 : j0 + HPC, :]
        in_maps.append({
            "xT": xT, "M": M, "bcol": bcol, "woE": woE,
            "zinit": np.zeros((HPC, BH), np.float32),
        })
    return in_maps


def _run(x, W_in, b_in, W_out, b_out, s_steps=S, trace=False):
    from concourse.bass_utils import run_bass_kernel_spmd

    key = ("nc", s_steps)
    if key not in _CACHE:
        _CACHE[key] = _build_nc(s_steps)
    nc = _CACHE[key]
    in_maps = _prep_inputs(x, W_in, b_in, W_out, s_steps)
    res = run_bass_kernel_spmd(nc, in_maps, list(range(NCORES)), trace=trace)
    logits = np.zeros((2, NB), np.float64)
    for core in range(NCORES):
        logits += res.results[core]["partial"].astype(np.float64)
    logits = logits.astype(np.float32).T + b_out.reshape(1, 2).astype(np.float32)
    # stable softmax, fp32
    m = logits.max(axis=-1, keepdims=True)
    e = np.exp(logits - m)
    probs = e / e.sum(axis=-1, keepdims=True)
    return probs.astype(np.float32), res


def kernel(x, W_in, b_in, W_out, b_out):
    probs, _ = _run(
        np.asarray(x), np.asarray(W_in), np.asarray(b_in), np.asarray(W_out),
        np.asarray(b_out),
    )
    return probs



# revision 17
# speedup vs baseline: 13.1463x; 13.1463x over previous
"""HazardRNN Trainium2 kernel — data-parallel, low-overhead edition.

Math (per batch lane n, hidden unit j):
    h_t[j,n] = tanh(W_in[j] * x[n,t] + b_in[j] + h_{t-1}[j,n]),  t = 0..S-1
    out[n]   = softmax(h_{S-1} @ W_out + b_out)

Sharding: pure data parallel. Each of the 8 cores owns 32 batch lanes and the
FULL hidden dim (800). Host->device traffic is therefore just the sharded x
(1 MB total) plus ~26 KB of replicated weights — the dominant cost of a call
is the axon RPC floor, not bytes.

Per-core layout: hidden j = g*100 + q for group g in 0..7, row q in 0..99.
Free (column) index f = g*32 + n packs (group, lane). Two accumulating fp32
matmuls per step compute all 800 hidden units for all 32 lanes:

  xr tile [16, 256/pos]: rows 0..7  block-diagonal x (row g holds x[n,t] in
                         free block g, zeros elsewhere)
                         rows 8..15 block-diagonal ones (bias carrier)
  WB [16,100] stationary: WB[g,q]=W_in[g*100+q], WB[8+g,q]=b_in[g*100+q]
  psum  = WB.T @ xr_t          (w*x + b, all groups at once)
  psum += I100 @ h_{t-1}       (h passthrough; skipped at t=0 since h_0=0)
  ACT: h_t = tanh(psum) -> h ping-pong tile, ready for the next step.

x is staged t-major into the block-diagonal rows by background DMA, CHUNK
steps per refill (8 DMAs, one per group row). The ones rows / off-diagonal
zeros / identity are built on-device by memsets — no host bytes.

Final projection: 8 accumulating matmuls (one per group) contract the full
800 hidden into logits [2, 32]; host adds b_out and applies softmax (256x2).

Sync: the ISA gives matmul/DMA/activation ONE wait slot, but the Tile
scheduler emits vector-clock wait lists of any length. A generic post pass
splits every multi-wait instruction: extra waits are hoisted into
single-wait InstDrains on the same engine immediately before it (engine
streams execute the merged block order, so semantics are identical).

The runner caches the jitted shard_map executable at module scope: warm calls
skip jax tracing/XLA compilation entirely (the dominant cost of the naive
run_bass_kernel_spmd path, which rebuilds the jit every call).
"""

import numpy as np

S = 1024
NB = 256        # total batch lanes (B*E)
NCORES = 8
LPC = NB // NCORES  # lanes per core = 32
G = 8           # hidden groups
HPG = 100       # hidden rows per group
HIDDEN = G * HPG
N = G * LPC     # moving free dim = 256
CHUNK = 64      # ring positions per x-refill
NCHUNKS = S // CHUNK

_CACHE: dict = {}


def _build_nc():
    import concourse.bass as bass
    import concourse.mybir as mybir
    from concourse.tile import TileContext

    f32 = mybir.dt.float32
    AF = mybir.ActivationFunctionType

    from concourse.masks import make_identity

    nc = bass.Bass()
    xT = nc.declare_dram_parameter("xT", [S, LPC], f32, isOutput=False)
    WBd = nc.declare_dram_parameter("WB", [2 * G, HPG], f32, isOutput=False)
    wod = nc.declare_dram_parameter("woG", [HPG, 2 * G], f32, isOutput=False)
    onesd = nc.declare_dram_parameter("ones", [CHUNK, LPC], f32, isOutput=False)
    outd = nc.declare_dram_parameter("partial", [2, LPC], f32, isOutput=True)

    with TileContext(nc) as tc:
        with (
            tc.tile_pool(name="const", bufs=1) as cp,
            tc.tile_pool(name="ring", bufs=1) as rp,
            tc.tile_pool(name="ps", bufs=4, space="PSUM") as pp,
            tc.tile_pool(name="ps_fin", bufs=1, space="PSUM") as pf,
            tc.tile_pool(name="fin", bufs=1) as fp,
        ):
            WBt = cp.tile([2 * G, HPG], f32, tag="WBt")
            woT = cp.tile([HPG, 2 * G], f32, tag="woT")
            IdB = cp.tile([HPG, HPG], f32, tag="IdB")
            zb = cp.tile([128, 1], f32, tag="zb")
            part = fp.tile([2, LPC], f32, tag="part")
            # x staging rings: only DMAs (+init memsets) ever write these.
            xr = [
                rp.tile([2 * G, CHUNK * N], f32, name=f"xr{i}", tag=f"xr{i}")
                for i in range(2)
            ]
            # h ping-pong: act(t) writes position (t+1)%2, mm_b reads t%2.
            hr = rp.tile([HPG, 2 * N], f32, name="hr", tag="hr")

            # ---- on-device init (before the DMAs that overwrite x rows) ----
            nc.vector.memzero(zb[:, :])
            make_identity(nc, IdB[:, :])
            for i in range(2):
                nc.vector.memzero(xr[i][:, :])

            # ---- DMAs ----
            nc.sync.dma_start(out=WBt[:], in_=WBd[:])
            nc.sync.dma_start(out=woT[:], in_=wod[:])

            def dma_row(buf, row, g, src):
                nc.sync.dma_start(
                    out=buf[row : row + 1, :]
                    .rearrange("p (t f) -> p t f", t=CHUNK)[
                        :, :, g * LPC : (g + 1) * LPC
                    ],
                    in_=src,
                )

            # block-diagonal ones rows (bias carrier), written once
            for i in range(2):
                for g in range(G):
                    dma_row(xr[i], G + g, g, onesd[:, :])

            def dma_x(c):
                buf = xr[c % 2]
                for g in range(G):
                    dma_row(buf, g, g, xT[c * CHUNK : (c + 1) * CHUNK, :])

            dma_x(0)
            dma_x(1)

            # ---- the scan (h_0 = 0, so step 0 has no h passthrough) ----
            for t in range(S):
                c, pos = divmod(t, CHUNK)
                buf = xr[c % 2]
                ps = pp.tile([128, N], f32, name="ps", tag="ps")
                nc.tensor.matmul(
                    out=ps[0:HPG, :],
                    lhsT=WBt[:, :],
                    rhs=buf[:, pos * N : (pos + 1) * N],
                    start=True,
                    stop=(t == 0),
                )
                if t > 0:
                    nc.tensor.matmul(
                        out=ps[0:HPG, :],
                        lhsT=IdB[:, :],
                        rhs=hr[:, (t % 2) * N : (t % 2 + 1) * N],
                        start=False,
                        stop=True,
                    )
                nc.scalar.activation(
                    out=hr[:, ((t + 1) % 2) * N : ((t + 1) % 2 + 1) * N],
                    in_=ps[0:HPG, :],
                    func=AF.Tanh,
                    bias=zb[0:HPG, :],
                )
                if pos == CHUNK - 1 and c + 2 < NCHUNKS:
                    dma_x(c + 2)

            # ---- final projection: logits[o, n] = sum_j W_out[j,o] h[j,n]
            ps2 = pf.tile([2, LPC], f32, name="ps2", tag="ps2")
            for g in range(G):
                nc.tensor.matmul(
                    out=ps2[:, :],
                    lhsT=woT[:, 2 * g : 2 * g + 2],
                    rhs=hr[:, (S % 2) * N + g * LPC : (S % 2) * N + (g + 1) * LPC],
                    start=(g == 0),
                    stop=(g == G - 1),
                )
            nc.vector.tensor_copy(part[:, :], ps2[:, :])
            nc.sync.dma_start(out=outd[:, :], in_=part[:, :])

    # ---- generic wait-splitting pass: every instruction keeps at most ONE
    # ISA wait; extra waits become single-wait InstDrains on the same engine
    # immediately before it. Engine streams follow merged block order, so
    # this is semantics-preserving for any instruction type.
    for bb in nc.m.functions[0].blocks:
        insts = list(bb.instructions)
        out_insts = []
        changed = False
        for i in insts:
            si = getattr(i, "sync_info", None)
            ws = None
            if si is not None:
                try:
                    ws = list(si.on_wait)
                except Exception:
                    ws = None
            if (
                ws is not None
                and len(ws) > 1
                and type(i).__name__ != "InstEventSemaphore"
            ):
                for k, w in enumerate(ws[:-1]):
                    d = mybir.InstDrain(
                        name=f"{i.name}_wsplit_{k}", ins=[], outs=[]
                    )
                    d.engine = i.engine
                    d.sync_info = type(si)(on_wait=[w], on_update=[])
                    nc.inst_map[d.name] = d
                    out_insts.append(d)
                si.on_wait = ws[-1:]
                changed = True
            out_insts.append(i)
        if changed:
            bb.instructions = out_insts

    # Build-time guard: nothing may carry more than one wait now.
    bad = []
    for bb in nc.m.functions[0].blocks:
        for i in bb.instructions:
            si = getattr(i, "sync_info", None)
            if si is None:
                continue
            try:
                nw = len(si.on_wait)
            except Exception:
                continue
            if nw > 1:
                bad.append(
                    (type(i).__name__, i.name,
                     [w.ant_name for w in si.on_wait])
                )
    if bad:
        raise RuntimeError(f"instructions with >1 ISA wait: {bad[:10]}")
    return nc


def _prep_concat(x, W_in, b_in, W_out):
    """Host-side shard prep: axis-0-concatenated per-core inputs, keyed by
    DRAM tensor name (the runner concatenates per-core shards on axis 0)."""
    w = W_in.reshape(HIDDEN).astype(np.float32)
    b = b_in.reshape(HIDDEN).astype(np.float32)
    wo = W_out.astype(np.float32)
    WB = np.empty((2 * G, HPG), np.float32)
    woG = np.empty((HPG, 2 * G), np.float32)
    for g in range(G):
        WB[g, :] = w[g * HPG : (g + 1) * HPG]
        WB[G + g, :] = b[g * HPG : (g + 1) * HPG]
        woG[:, 2 * g : 2 * g + 2] = wo[g * HPG : (g + 1) * HPG, :]
    ones = np.ones((CHUNK, LPC), np.float32)
    # [NCORES*S, LPC]: core c's shard is x[c*32:(c+1)*32, :] transposed t-major
    xTcat = np.ascontiguousarray(
        x.reshape(NCORES, LPC, S).astype(np.float32).transpose(0, 2, 1)
    ).reshape(NCORES * S, LPC)
    return {
        "xT": xTcat,
        "WB": np.tile(WB, (NCORES, 1)),
        "woG": np.tile(woG, (NCORES, 1)),
        "ones": np.tile(ones, (NCORES, 1)),
    }


def _get_runner():
    """Build the Bass module and a CACHED jitted shard_map executable."""
    if "runner" in _CACHE:
        return _CACHE["runner"]
    import jax
    import concourse.mybir as mybir
    from jax.sharding import Mesh, PartitionSpec
    from jax.experimental.shard_map import shard_map
    from concourse.bass2jax import (
        _bass_exec_p, install_neuronx_cc_hook, partition_id_tensor,
    )

    nc = _CACHE.get("nc")
    if nc is None:
        nc = _CACHE["nc"] = _build_nc()
    install_neuronx_cc_hook()

    partition_name = (
        nc.partition_id_tensor.name if nc.partition_id_tensor else None
    )
    in_names, out_names, out_avals, zero_outs = [], [], [], []
    for alloc in nc.m.functions[0].allocations:
        if not isinstance(alloc, mybir.MemoryLocationSet):
            continue
        name = alloc.memorylocations[0].name
        if alloc.kind == "ExternalInput":
            if name != partition_name:
                in_names.append(name)
        elif alloc.kind == "ExternalOutput":
            out_names.append(name)
            shape = tuple(alloc.tensor_shape)
            dtype = mybir.dt.np(alloc.dtype)
            out_avals.append(jax.core.ShapedArray(shape, dtype))
            zero_outs.append(np.zeros(shape, dtype))
    n_params = len(in_names)
    n_outs = len(out_avals)
    in_names_full = in_names + out_names
    if partition_name is not None:
        in_names_full.append(partition_name)

    donate = tuple(range(n_params, n_params + n_outs))

    def _body(*args):
        operands = list(args)
        if partition_name is not None:
            operands.append(partition_id_tensor())
        outs = _bass_exec_p.bind(
            *operands,
            out_avals=tuple(out_avals),
            in_names=tuple(in_names_full),
            out_names=tuple(out_names),
            lowering_input_output_aliases=(),
            sim_require_finite=True,
            sim_require_nnan=True,
            nc=nc,
        )
        return tuple(outs)

    devices = jax.devices()[:NCORES]
    mesh = Mesh(np.asarray(devices), ("core",))
    in_specs = (PartitionSpec("core"),) * (n_params + n_outs)
    out_specs = (PartitionSpec("core"),) * len(out_names)
    sharded = jax.jit(
        shard_map(
            _body, mesh=mesh, in_specs=in_specs,
            out_specs=out_specs, check_rep=False,
        ),
        donate_argnums=donate,
        keep_unused=True,
    )

    runner = (sharded, in_names, out_names, zero_outs)
    _CACHE["runner"] = runner
    return runner


def _run_hw(concat_map):
    sharded, in_names, out_names, zero_outs = _get_runner()
    concat_in = [concat_map[nm] for nm in in_names]
    concat_zeros = [
        np.zeros((NCORES * z.shape[0], *z.shape[1:]), z.dtype)
        for z in zero_outs
    ]
    out_arrs = sharded(*concat_in, *concat_zeros)
    name_to_arr = dict(zip(out_names, out_arrs))
    pa = np.asarray(name_to_arr["partial"]).reshape(NCORES, 2, LPC)
    return pa


def _postprocess(pa, b_out):
    # pa: [NCORES, 2, LPC] complete logits per core (hidden fully on-core)
    logits = np.transpose(pa, (0, 2, 1)).reshape(NB, 2)
    logits = logits + b_out.reshape(1, 2).astype(np.float32)
    m = logits.max(axis=-1, keepdims=True)
    e = np.exp(logits - m)
    return (e / e.sum(axis=-1, keepdims=True)).astype(np.float32)


def kernel(x, W_in, b_in, W_out, b_out):
    x = np.asarray(x)
    concat_map = _prep_concat(
        x, np.asarray(W_in), np.asarray(b_in), np.asarray(W_out)
    )
    pa = _run_hw(concat_map)
    return _postprocess(pa, np.asarray(b_out))
